# revision 23
# baseline (speedup 1.0000x reference)
"""CapsuleLayer kernel for Trainium2 (8 NeuronCores, data-parallel over batch).

Math: the reference's dynamic-routing loop is degenerate — `delta` is summed
over the capsule axis and broadcast back, so the logits stay constant across
capsules and softmax stays uniform (1/16) for all 3 iterations. The module
therefore reduces exactly to

    t   = (conv2d(x, sum_c W[c]) + sum_c b[c]) / 16      # 16-out-channel conv
    out = t * |t| / (1 + t*t)                            # scalar squash

The capsule sum is folded into the conv weights on the host (conv is linear in
the weights), leaving a [O=16, I=64, 3, 3] VALID conv + pointwise epilogue.

Device strategy per core (8 images per core):
  - x for an image pair lives in SBUF as [128, 66, 66] (partitions = parity*64
    + in_channel), loaded with one contiguous 2.2 MB DMA.
  - The conv runs on the TensorEngine as 9 accumulating matmuls (one per
    3x3 tap, shifts expressed in the rhs access pattern), packed 8-wide into
    the 128x128 array with tile_position (2 row groups x 4 col groups,
    K=64, M=16 tiles).  PSUM tile [128, 2048] = 4 banks holds one image
    pair: bank = parity*2 + h_group, partitions 32j+o (o<16) = 4 h-tiles x
    16 ch (partitions 32j+16..32j+31 are never written).
  - Epilogue (3 engine passes, running in the PE shadow; ACT and DVE read
    PSUM independently so the sign and magnitude paths overlap):
      ACT : sgn = Sign(ps + bias)               -> bf16 (+-1, exact)
      DVE : v   = SQUASH_V_ANT(ps)              -> bf16  (custom 8-slice op:
            w = ps^2+1; seed = bitwise_not(w)*c0; y1 = one-Newton 1/w;
            v = 1 - y1  ==  t^2/(1+t^2), max abs err ~1.7e-3; b==0 fast
            path -- with b!=0 an Identity pass materializes t first)
      DVE : f   = v * sgn  (bf16 tensor_tensor, 2x mode)
  - One [128, 2048] out-DMA per pair via GpSimd SWDGE (per-dma_start issue
    occupancy makes partition-sliced compact writes and HWDGE-ring sharing a
    net loss); the host drops the garbage partition halves.  PSUM runs as
    [128, 1024] units (bank = 2*hg + rg) with 4 buffers so the PE can run ~3
    units ahead of the epilogue's PSUM release; x is double-half-DMA'd with
    a 4-deep ring.
"""

import numpy as np

N_CORES = 8
B_PER_CORE = 8  # 64 images / 8 cores

# Chebyshev-minimax pair for the bitwise-not reciprocal seed (same constants
# as concourse's RECIPROCAL_APPROX_FAST).
_CHEB_C0 = -0.23549792
_CHEB_C1 = 2.0017324

_SQUASH_NAME = "SQUASH_V_ANT"


def _get_squash_op():
    """Author + register the fused squash-magnitude DVE op (idempotent).

    body: v = 1 - y1,  y1 = y0*(c1 - w*y0),  y0 = bitwise_not(w)*c0,
          w = Src0^2 + 1      — exactly 8 ALU stages.
    """
    import concourse.dve_ops as dve_ops

    for o in dve_ops.OPS:
        if o.name == _SQUASH_NAME:
            return o
    from concourse.dve_spec import AluOp, Bin, C0, C1, One, Spec, Src0, lower, sq
    from concourse.dve_uop import DveOpSpec

    w = sq(Src0) + One
    n = Bin(AluOp.BITWISE_NOT, w, w)
    y0 = n * C0
    y1 = y0 * (C1 - w * y0)
    body = One - y1

    def _ref(in0, in1, c0, c1, c2):
        x = np.asarray(in0).astype(np.float32)
        w = x * x + np.float32(1.0)
        nn = (~w.view(np.int32)).view(np.float32)
        y0 = nn * np.float32(c0)
        y1 = y0 * (np.float32(c1) - w * y0)
        return np.float32(1.0) - y1

    spec = Spec(body=body, reference=_ref)
    row = dve_ops._CUSTOM_DVE_ROW_BASE + len(dve_ops.OPS)
    dve_ops._SUB_OPCODE_FOR_NAME[_SQUASH_NAME] = row
    shas = {
        ver: DveOpSpec(
            name=_SQUASH_NAME, opcode=row, uops=lower(spec, ver=ver), rd1_en=False
        ).sha(ver)
        for ver in ("v3", "v4")
    }
    op = dve_ops.DveOp(_SQUASH_NAME, spec, subdim=False, uops_sha=shas)
    dve_ops.OPS.append(op)
    dve_ops.CUSTOM_DVE_SPECS[_SQUASH_NAME] = spec
    return op


def _build_nc(
    repeat=1,
    loop_repeat=1,
    conv_bf16=True,
    parts=None,
    x_bufs=4,
    stagger=False,
    bias_zero=True,
):
    # parts: subset of {"in", "mm", "epi", "out"} for bench attribution;
    # None = all. "cal" alone = loop-overhead calibration body.
    if parts is None:
        parts = {"in", "mm", "epi", "out"}
    import contextlib

    import concourse.bacc as bacc
    import concourse.mybir as mybir
    import concourse.tile as tile

    squash_op = _get_squash_op()

    f32 = mybir.dt.float32
    i16 = mybir.dt.int16
    cdt = mybir.dt.bfloat16 if conv_bf16 else f32
    # Bacc (not raw Bass): its finalize() runs move_matmul_waits_to_ldweights
    # + generate_event_semaphores, required for TRN2's 1-wait-per-instruction
    # limit (our first matmuls collect several Tile sem waits).
    nc = bacc.Bacc(None, target_bir_lowering=False, debug=False)

    x_d = nc.dram_tensor("x", [512, 66, 66], cdt, kind="ExternalInput")
    w_d = nc.dram_tensor("w", [128, 288], cdt, kind="ExternalInput")
    bv_d = nc.dram_tensor("bvec", [128, 1], f32, kind="ExternalInput")
    # Raw per-pair dump [pair, partition, bank*512]; host drops the garbage
    # partition halves. One big DMA per pair: per-dma_start issue occupancy
    # on the HWDGE ring (~1us each) makes 4 sliced DMAs a net loss.
    out_d = nc.dram_tensor("out", [4, 128, 2048], cdt, kind="ExternalOutput")

    with tile.TileContext(nc) as tc:
        with (
            tc.tile_pool(name="const", bufs=1) as cp,
            tc.tile_pool(name="xp", bufs=x_bufs) as xp,
            tc.tile_pool(name="psp", bufs=4, space="PSUM") as psp,
            tc.tile_pool(name="wk", bufs=3) as wk,
        ):
            w_t = cp.tile([128, 288], cdt)
            nc.sync.dma_start(out=w_t[:, :], in_=w_d[:, :])
            b_t = cp.tile([128, 1], f32)
            nc.sync.dma_start(out=b_t[:, :], in_=bv_d[:, :])

            use_stagger = stagger and loop_repeat > 1 and parts != {"cal"}
            if loop_repeat > 1:  # bench only: HW loop repeating the body
                loop_cm = tc.For_i(
                    0,
                    loop_repeat,
                    1,
                    hint_engines=(
                        mybir.EngineType.PE,
                        mybir.EngineType.Activation,
                        mybir.EngineType.DVE,
                        mybir.EngineType.SP,
                    ),
                    staggered_reset=use_stagger,
                )
            else:
                loop_cm = contextlib.nullcontext()
            with loop_cm:
              if parts == {"cal"}:
                cal_t = wk.tile([128, 16], f32, tag="cal")
                nc.vector.memset(cal_t[:, :], 0.0)
              pending_out = []  # (pair_idx, f_tile) awaiting their out-DMAs

              def flush_out(nc=nc, out_d=out_d):
                  while pending_out:
                      fp, f = pending_out.pop(0)
                      # SWDGE on the otherwise-idle GpSimd engine: keeps
                      # both HWDGE rings (SP: x prefetch, ACT: Sign) free
                      nc.gpsimd.dma_start(out=out_d[fp, :, :], in_=f[:, :])

              n_p4 = 0 if parts == {"cal"} else 4 * repeat
              for p4 in range(n_p4):
                p = p4 % 4
                if use_stagger and p4 % (n_p4 // 4) == 0 and p4 > 0:
                    tc.stage_boundary()
                x_t = xp.tile([128, 66, 66], cdt, tag="x")
                if "in" in parts:
                    # two half-DMAs: hg=0 matmuls only need rows < 34, so the
                    # PE can start as soon as the first half lands
                    nc.sync.dma_start(
                        out=x_t[:, 0:34, :], in_=x_d[128 * p : 128 * (p + 1), 0:34, :]
                    )
                    # second half on the ACT HWDGE ring: both halves
                    # transfer concurrently (ACT's Sign work is post-MM, so
                    # its DMA occupancy sits in the pre-MM window)
                    nc.scalar.dma_start(
                        out=x_t[:, 34:66, :], in_=x_d[128 * p : 128 * (p + 1), 34:66, :]
                    )
                elif "inx" in parts:
                    # bench probe: same waits/sems as "in" but ~no HBM traffic
                    nc.vector.memset(x_t[:, 0:1, 0:1], 0.0)
                    nc.sync.dma_start(
                        out=x_t[:, 0:2, :], in_=x_d[128 * p : 128 * (p + 1), 0:2, :]
                    )
                elif "mm" in parts or "ind" in parts:
                    # bench only: touch-write so Tile allocates the tile
                    nc.vector.memset(x_t[:, 0:1, 0:1], 0.0)
                if "ind" in parts:
                    # bench probe: full-size DMA traffic, uncoupled from MMs
                    xd_t = xp.tile([128, 66, 66], cdt, tag="xd")
                    nc.sync.dma_start(
                        out=xd_t[:, :, :], in_=x_d[128 * p : 128 * (p + 1), :, :]
                    )
                if "out" in parts:
                    flush_out()
                # f collects both hg halves of a pair; written out as one DMA
                f = None
                if "epi" in parts:
                    f = wk.tile([128, 2048], cdt, tag="f")
                # One PSUM unit = 2 contiguous banks = one hg half of a pair
                # (bank = 2*hg + rg).  bufs=4 doubles the pipeline lookahead
                # vs a single [128, 2048] pair tile: the PE can run up to 3
                # units (~5.8us) ahead of the epilogue's PSUM release.
                for hg in range(2):
                    ps = psp.tile([128, 1024], f32, tag="ps")
                    if "mm" not in parts and "epi" in parts:
                        nc.vector.memset(ps[:, 0:1], 0.0)  # bench only
                    if "mm" in parts:
                        for t in range(9):
                            kh, kw = divmod(t, 3)
                            for rg in range(2):
                                for j in range(4):
                                    h0 = (hg * 4 + j) * 8
                                    nc.tensor.matmul(
                                        ps[
                                            32 * j : 32 * j + 16,
                                            512 * rg : 512 * rg + 512,
                                        ],
                                        w_t[
                                            64 * rg : 64 * rg + 64,
                                            32 * t : 32 * t + 16,
                                        ],
                                        x_t[
                                            64 * rg : 64 * rg + 64,
                                            h0 + kh : h0 + kh + 8,
                                            kw : kw + 64,
                                        ],
                                        start=(t == 0),
                                        stop=(t == 8),
                                        tile_position=(64 * rg, 32 * j),
                                        skip_group_check=True,
                                    )

                    if "epi" in parts:
                        sgn = wk.tile([128, 1024], cdt, tag="sgn")
                        v = wk.tile([128, 1024], cdt, tag="v")
                        # sgn = sign(ps + bias) in {-1, 0, +1} (bf16 exact)
                        nc.scalar.activation(
                            sgn[:, :],
                            ps[:, :],
                            mybir.ActivationFunctionType.Sign,
                            bias=b_t[:, 0:1],
                        )
                        if bias_zero:
                            # v = ps^2/(1+ps^2) from PSUM (b==0 fast path)
                            sq_in = ps[:, :]
                        else:
                            # general path: materialize t = ps + bias first
                            tb = wk.tile([128, 1024], cdt, tag="tb")
                            nc.scalar.activation(
                                tb[:, :],
                                ps[:, :],
                                mybir.ActivationFunctionType.Identity,
                                bias=b_t[:, 0:1],
                            )
                            sq_in = tb[:, :]
                        nc.vector._custom_dve(
                            squash_op,
                            out=v[:, :],
                            in0=sq_in,
                            s0=_CHEB_C0,
                            s1=_CHEB_C1,
                        )
                        # f half = v * sgn  (bf16 tensor_tensor, 2x mode)
                        nc.vector.tensor_mul(
                            f[:, 1024 * hg : 1024 * hg + 1024], v[:, :], sgn[:, :]
                        )
                if "epi" in parts:
                    pending_out.append((p, f))
              if "out" in parts and "epi" in parts:
                  flush_out()
    # Run the Bacc pass pipeline (wait splitting, reg alloc, ...) now; the
    # axon/pjrt execute path binds the primitive without finalizing.
    nc.finalize()
    return nc


def _np_bf16(a):
    import ml_dtypes

    return np.ascontiguousarray(a.astype(ml_dtypes.bfloat16))


def _prep_weights(W, b):
    """[16,16,64,3,3] capsule weights -> [128, 288] lhsT blocks (pre-summed
    over capsules, /16 for the uniform routing probs, duplicated into both
    partition halves).  Bias -> [128, 1] per-partition vector."""
    Wsum = np.asarray(W, dtype=np.float32).sum(axis=0) / 16.0  # [16, 64, 3, 3]
    w_arr = np.zeros((128, 288), np.float32)
    for t in range(9):
        kh, kw = divmod(t, 3)
        blk = np.ascontiguousarray(Wsum[:, :, kh, kw].T)  # [64 in, 16 out]
        w_arr[0:64, 32 * t : 32 * t + 16] = blk
        w_arr[64:128, 32 * t : 32 * t + 16] = blk
    bsum = np.asarray(b, dtype=np.float32).sum(axis=0) / 16.0  # [16]
    bvec = np.zeros((128, 1), np.float32)
    for j in range(4):
        bvec[32 * j : 32 * j + 16, 0] = bsum
    return w_arr, bvec


def make_in_maps(x, W, b, conv_bf16=True):
    x = np.ascontiguousarray(np.asarray(x, dtype=np.float32))
    w_arr, bvec = _prep_weights(W, b)
    if conv_bf16:
        x = _np_bf16(x)
        w_arr = _np_bf16(w_arr)
    return [
        {
            "x": np.ascontiguousarray(
                x[c * B_PER_CORE : (c + 1) * B_PER_CORE].reshape(512, 66, 66)
            ),
            "w": w_arr,
            "bvec": bvec,
        }
        for c in range(N_CORES)
    ]


def gather_out(per_core_outs):
    """Unshuffle raw [4, 128, 2048] per-core dumps into [64, 65536, 1] f32.

    partition = 32*j + oo (oo<16 valid); free = 512*bank + n, bank = 2*hg+rg;
    out[b=2p+rg, oo*4096 + (hg*4+j)*512 + n]."""
    full = np.empty((64, 65536), np.float32)
    for c, raw in enumerate(per_core_outs):
        r = np.asarray(raw, dtype=np.float32).reshape(4, 4, 32, 2, 2, 512)
        v = r[:, :, :16].transpose(0, 4, 2, 3, 1, 5)  # [p, rg, oo, hg, j, n]
        full[c * 8 : (c + 1) * 8] = v.reshape(8, 65536)
    return full.reshape(64, 65536, 1)


def kernel(x, W, b):
    from concourse.bass_utils import run_bass_kernel_spmd

    bias_zero = not np.any(np.asarray(b, dtype=np.float32))
    nc = _build_nc(conv_bf16=True, bias_zero=bias_zero)
    in_maps = make_in_maps(x, W, b, conv_bf16=True)
    res = run_bass_kernel_spmd(nc, in_maps, list(range(N_CORES)))
    return gather_out([res.results[c]["out"] for c in range(N_CORES)])
